# revision 37
# baseline (speedup 1.0000x reference)
"""Trainium2 Bass kernel for causal multi-head self-attention.

Problem: nn_MultiHeadSelfAttention (B=2, T=2048, D=768, H=12, HD=64).

    qkv = x @ Wqkv ; per-head causal softmax(q k^T / sqrt(hd)) @ v ; out @ Wo + bo

Sharding (8 cores): data-parallel over B (2) x tensor-parallel over heads
(4 groups of 3 heads).  Each core computes the QKV projection for its own
head slice, runs attention for its 3 heads, and produces a partial o_proj
output [T, D] (rows of Wo for its heads).  Host sums the 4 partials per
batch and adds the bias.

v2 design notes (HAM-warmth + PE row-tiling):
  - Scores (K=64 contraction) are emitted as row-tile PAIRS: head0 on PE
    rows 0:63 and head1 on rows 64:127 run concurrently (the bass
    auto-derived tile_position comes straight from the operand base
    partitions, which the [Qh0|Qh1]/[Kh0|Kh1] projection layout already
    provides).  Head2 pairs even-kj (operands re-based to partitions 0:63)
    with odd-kj (operands at partitions 64:127 via one SBUF->SBUF DMA).
  - P^T for all 3 heads persists in SBUF (bf16), so PV runs q-window-major:
    one PSUM bank per (head, window) accumulates over all its key tiles and
    is drained immediately.  o_proj for a window runs as soon as its 3 OT
    slices land, spreading the output DMA across the kernel.
  - The softmax division is applied to OT itself: the denominator row is
    inverted (DVE), partition-broadcast (GPSIMD), and multiplied into the
    PSUM->SBUF drain, so o_proj is a pure PSUM accumulation (head0+head2
    chained on PE rows 0:63 concurrent with head1(+head2 odd) on 64:127)
    and the per-head scalar_tensor_tensor epilogue disappears.
  - exp is the ACT-engine bottleneck (~52k columns); scores emission is
    interleaved with the m2/V projections so the PE always has independent
    work while ACT drains the score PSUM tiles (keeps the HAM clock gate
    at K=8/8 instead of the re-throttled 1.2 GHz the v1 kernel sat at).
"""

import os
import sys

for _p in ("/opt/trn_rl_repo",):
    if os.path.isdir(_p) and _p not in sys.path:
        sys.path.insert(0, _p)

import numpy as np
import ml_dtypes

import concourse.bass as bass
import concourse.mybir as mybir
import concourse.tile as tile
from concourse import bacc
from concourse.bass_utils import run_bass_kernel_spmd
from concourse.masks import make_lower_triangular

F32 = mybir.dt.float32

MM_MODE = os.environ.get("MM_DT", "bf16")
if MM_MODE == "f32r":
    MM_DT = mybir.dt.float32r
    NP_IN = np.float32
else:
    MM_DT = mybir.dt.bfloat16
    NP_IN = ml_dtypes.bfloat16

B, T, D, H = 2, 2048, 768, 12
HD = 64
HPC = 3            # heads per core
GROUPS = 4         # head groups (tensor-parallel)
N_CORES = 8
KT = D // 128      # 6 k-tiles over the feature dim
QKCOLS = HPC * 2 * HD  # 384 projected q/k columns
VC = HPC * HD          # 192 v columns
SCALE = 1.0 / np.sqrt(HD)
NCHUNK = 512
NQW = T // NCHUNK  # 4 query windows
NKJ = T // 128     # 16 key tiles
VBW = HD + 2       # v block width incl. ones columns
NEG = -1.0e30

# P^T column offset per key tile (ragged causal extents packed back to back)
EXT = [T - 128 * kj for kj in range(NKJ)]
OFF = [2048 * kj - 64 * kj * (kj - 1) for kj in range(NKJ)]
PTW = OFF[NKJ - 1] + EXT[NKJ - 1]  # 17408


def _chunks(kj):
    """(qstart, width) score chunks for key tile kj (causal: q >= 128*kj)."""
    out = []
    q = 128 * kj
    while q < T:
        w = min(NCHUNK, T - q)
        out.append((q, w))
        q += w
    return out


_CACHE = {}


def _one_act_table():
    """Make the act-table-load pass resolve both Exp and Ln to the single
    set that holds them both (natural_log_exp_and_others).  The pass picks
    a canonical set per function, so a kernel mixing exp and ln otherwise
    thrashes ACT_TABLE_LOADs (~1.3us each).  Only the python-side
    membership view is narrowed — set ids keep their act_info.json order,
    and the chosen set genuinely contains both functions."""
    import concourse.hw_specs as hw_specs
    if _CACHE.get("act_patched"):
        return
    orig = hw_specs.get_activation_tables

    def patched(arch):
        t = {name: set(fn) for name, fn in orig(arch).items()}
        both = {mybir.ActivationFunctionType.Exp,
                mybir.ActivationFunctionType.Ln}
        keep = "natural_log_exp_and_others"
        if keep in t and both <= t[keep]:
            for name, fns in t.items():
                if name != keep:
                    fns -= both
        return t

    hw_specs.get_activation_tables = patched
    bacc.get_activation_tables = patched
    _CACHE["act_patched"] = True


def _build_program():
    _one_act_table()
    nc = bacc.Bacc("TRN2", target_bir_lowering=False, debug=False,
                   num_devices=N_CORES, name="mhsa")

    xT_d = nc.dram_tensor("xT", [D, T], MM_DT, kind="ExternalInput").ap()
    wqk_d = nc.dram_tensor("wqk", [D, QKCOLS], MM_DT, kind="ExternalInput").ap()
    wv_d = nc.dram_tensor("wv", [D, VC], MM_DT, kind="ExternalInput").ap()
    wo_d = nc.dram_tensor("wo", [VC, D], MM_DT, kind="ExternalInput").ap()
    out_d = nc.dram_tensor("out", [T, D], F32, kind="ExternalOutput").ap()

    with tile.TileContext(nc) as tc:
        with (
            tc.tile_pool(name="const", bufs=1) as const,
            tc.tile_pool(name="persist", bufs=1) as persist,
            tc.tile_pool(name="obp", bufs=3) as obp,
            tc.tile_pool(name="nrm", bufs=2) as nrm,
            tc.tile_pool(name="psc", bufs=2, space="PSUM") as psc,
            tc.tile_pool(name="pmix", bufs=2, space="PSUM") as pmix,
            tc.tile_pool(name="pacc", bufs=2, space="PSUM") as pacc,
        ):
            # ---- constants ----
            maskneg = const.tile([128, 128], F32, tag="maskneg")
            make_lower_triangular(nc, maskneg, val=NEG, diag=False)
            ones_f = const.tile([128, 2], F32, tag="ones_f")
            nc.gpsimd.memset(ones_f, 1.0)
            ones_t = const.tile([128, 2], MM_DT, tag="ones_t")
            nc.vector.tensor_copy(ones_t, ones_f)

            # ---- input tiles (k-interleaved so matmul k=0 starts early) ----
            # xT loads are split in column halves: all first halves (plus
            # wqk) land in ~4.5us, so the n0-outer projection below starts
            # ~6us earlier than waiting for the full 3.5MB input stream
            xT_t, wqk_t = [], []
            for k in range(KT):
                xt = persist.tile([128, T], MM_DT, tag=f"xT{k}")
                nc.sync.dma_start(xt[:, 0:T // 2],
                                  xT_d[k * 128:(k + 1) * 128, 0:T // 2])
                xT_t.append(xt)
                wt = persist.tile([128, QKCOLS], MM_DT, tag=f"wqk{k}")
                nc.sync.dma_start(wt, wqk_d[k * 128:(k + 1) * 128, :])
                wqk_t.append(wt)
            for k in range(KT):
                nc.sync.dma_start(xT_t[k][:, T // 2:T],
                                  xT_d[k * 128:(k + 1) * 128, T // 2:T])
            # wv/wo tiles allocated here; their loads are emitted after the
            # m01 projection DMAs so xT/wqk win the queue and matmul 0
            # starts sooner (wv is first read mid-phase-0, wo at ops(0))
            wv_t = []
            for k in range(KT):
                wt = persist.tile([128, VC], MM_DT, tag=f"wv{k}", name=f"wv{k}")
                wv_t.append(wt)
            # o_proj weights: h0 rows on partitions 0:64, h1 on 64:128,
            # h2 on both 0:64 (wo2, even token tiles) and 64:128 (wo2b, odd)
            wo01 = persist.tile([128, D], MM_DT, tag="wo01")
            wo2 = persist.tile([64, D], MM_DT, tag="wo2")
            wo2b = persist.tile([128, D], MM_DT, tag="wo2b")

            def load_wv():
                for k in range(KT):
                    nc.sync.dma_start(wv_t[k], wv_d[k * 128:(k + 1) * 128, :])

            def load_wo():
                nc.sync.dma_start(wo01[0:64, :], wo_d[0:HD, :])
                nc.sync.dma_start(wo01[64:128, :], wo_d[HD:2 * HD, :])
                nc.sync.dma_start(wo2, wo_d[2 * HD:3 * HD, :])
                nc.sync.dma_start(wo2b[64:128, :], wo_d[2 * HD:3 * HD, :])

            # ---- persistent intermediates ----
            mt = [persist.tile([128, T], MM_DT, tag=f"mt{m}", name=f"mt{m}")
                  for m in range(3)]
            kt2 = persist.tile([64, T], MM_DT, tag="kt2")     # Kh2 -> base 0
            qk2b = persist.tile([128, T], MM_DT, tag="qk2b")  # Qh2 -> base 64
            # V blocks: [128, (j, h, VBW)] — one strided copy lands all 3
            # heads of a token tile; ones columns at 64:66 of each block
            V_all = persist.tile([128, NKJ * HPC * VBW], MM_DT, tag="V")
            va3 = V_all.rearrange("p (b c) -> p b c", c=VBW)
            nc.vector.tensor_copy(
                va3[:, :, HD:HD + 2],
                ones_t.unsqueeze(1).to_broadcast((128, NKJ * HPC, 2)))
            PT = [persist.tile([128, PTW], MM_DT, tag=f"PT{h}", name=f"PT{h}")
                  for h in range(HPC)]
            OT01 = persist.tile([128, T], MM_DT, tag="OT01", name="OT01")
            OT2 = persist.tile([64, T], MM_DT, tag="OT2", name="OT2")
            OT2b = persist.tile([128, T], MM_DT, tag="OT2b", name="OT2b")

            # ---- unit emitters ----
            def proj_unit(m, n0):
                ps = pmix.tile([128, NCHUNK], F32, tag="mix", name="proj")
                for k in range(KT):
                    nc.tensor.matmul(
                        ps, lhsT=wqk_t[k][:, m * 128:(m + 1) * 128],
                        rhs=xT_t[k][:, n0:n0 + NCHUNK],
                        start=(k == 0), stop=(k == KT - 1))
                nc.vector.tensor_copy(mt[m][:, n0:n0 + NCHUNK], ps)

            def vproj_unit(j):
                ps = pmix.tile([128, NCHUNK], F32, tag="mix", name="vproj")
                for k in range(KT):
                    nc.tensor.matmul(
                        ps[:, :VC], lhsT=xT_t[k][:, j * 128:(j + 1) * 128],
                        rhs=wv_t[k], start=(k == 0), stop=(k == KT - 1))
                v3 = V_all[:, j * HPC * VBW:(j + 1) * HPC * VBW].rearrange(
                    "p (h c) -> p h c", c=VBW)
                p3 = ps[:, :VC].rearrange("p (h c) -> p h c", c=HD)
                nc.vector.tensor_copy(v3[:, :, 0:HD], p3)

            def sc_block(h, lhsT, rhs_src, lo, kj, blk):
                """One exp block for head h / key tile kj: up to 2 score
                matmuls (bank-aligned halves of a 2-bank PSUM tile) drained
                by a single ACTIVATE — halves the ACT per-instr overhead."""
                wtot = sum(w for _, w in blk)
                ps = psc.tile([128, 2 * NCHUNK], F32, tag="sc", name="sc")
                off = 0
                for q, w in blk:
                    nc.tensor.matmul(ps[:, off:off + w],
                                     lhsT=lhsT[lo:lo + 64,
                                               kj * 128:kj * 128 + 128],
                                     rhs=rhs_src[lo:lo + 64, q:q + w],
                                     start=True, stop=True)
                    if q == 128 * kj:
                        nc.vector.tensor_add(ps[:, off:off + 128],
                                             ps[:, off:off + 128], maskneg)
                    off += w
                c = OFF[kj] + (blk[0][0] - 128 * kj)
                nc.scalar.activation(PT[h][:, c:c + wtot], ps[:, :wtot],
                                     mybir.ActivationFunctionType.Exp,
                                     scale=float(SCALE))

            def _blocks(kj):
                ch = _chunks(kj)
                return [ch[i:i + 2] for i in range(0, len(ch), 2)]

            def sc01_unit(kj, blk):
                # head0 on PE rows 0:63, head1 on rows 64:127 (concurrent)
                sc_block(0, mt[1], mt[0], 0, kj, blk)
                sc_block(1, mt[1], mt[0], 64, kj, blk)

            def sc2_units():
                """Head2 score pair units: even kj at rows 0:63 (kt2/mt2),
                odd kj at rows 64:127 (mt2/qk2b), block i with block i."""
                wins = [[] for _ in range(NQW)]
                for kp in range(0, NKJ, 2):
                    ba, bb = _blocks(kp), _blocks(kp + 1)
                    for i in range(max(len(ba), len(bb))):
                        def emit(kp=kp, i=i, ba=ba, bb=bb):
                            if i < len(ba):
                                sc_block(2, kt2, mt[2], 0, kp, ba[i])
                            if i < len(bb):
                                sc_block(2, mt[2], qk2b, 64, kp + 1, bb[i])
                        wins[kp // 4].append(emit)
                return wins

            def pv_chain(h, qw):
                acc = pacc.tile([128, NCHUNK], F32, tag="acc", name="acc")
                q0 = NCHUNK * qw
                last = 4 * qw + 3
                for kj in range(last + 1):
                    lo = max(0, 128 * kj - q0)
                    c = OFF[kj] + (q0 + lo - 128 * kj)
                    vb = (kj * HPC + h) * VBW
                    nc.tensor.matmul(
                        acc[:VBW, lo:NCHUNK],
                        lhsT=V_all[:, vb:vb + VBW],
                        rhs=PT[h][:, c:c + (NCHUNK - lo)],
                        start=(kj == 0), stop=(kj == last))
                return acc

            def norm_unit(h, qw, acc):
                """OT[:, window] = acc[0:64] / denominator-row (pre-divided
                so o_proj can accumulate heads directly in PSUM)."""
                # Stage acc (attention rows + denominator row) out of PSUM
                # right away so the bank frees without waiting on the recip
                # chain.  1/d runs on ACT as exp(-ln d) — the DVE reciprocal
                # is ~6.5ns/element (3.3us per row) and the custom-DVE
                # reciprocal_approx_fast produces garbage on this toolchain.
                # _one_act_table() keeps ln+exp in a single table set.
                stage = nrm.tile([65, NCHUNK], F32, tag="stage")
                nc.vector.tensor_copy(stage, acc[0:65, :])
                rl = nrm.tile([1, NCHUNK], F32, tag="rl")
                nc.scalar.activation(rl, stage[64:65, :],
                                     mybir.ActivationFunctionType.Ln)
                rr = nrm.tile([1, NCHUNK], F32, tag="rr")
                nc.scalar.activation(rr, rl,
                                     mybir.ActivationFunctionType.Exp,
                                     scale=-1.0)
                rb = nrm.tile([64, NCHUNK], F32, tag="rb")
                nc.gpsimd.partition_broadcast(rb, rr, channels=64)
                qs = qw * NCHUNK
                if h == 0:
                    nc.vector.tensor_mul(OT01[0:64, qs:qs + NCHUNK],
                                         stage[0:64, :], rb)
                elif h == 1:
                    st = nrm.tile([64, NCHUNK], MM_DT, tag="st")
                    nc.vector.tensor_mul(st, stage[0:64, :], rb)
                    nc.sync.dma_start(OT01[64:128, qs:qs + NCHUNK], st)
                else:
                    nc.vector.tensor_mul(OT2[:, qs:qs + NCHUNK],
                                         stage[0:64, :], rb)
                    nc.sync.dma_start(OT2b[64:128, qs:qs + NCHUNK],
                                      OT2[:, qs:qs + NCHUNK])

            def op_unit(tt):
                ob = obp.tile([128, D], F32, tag="ob")
                ts = tt * 128
                even = (tt % 2 == 0)
                for n0, nw in ((0, 512), (512, 256)):
                    p0 = pmix.tile([128, NCHUNK], F32, tag="mix", name="po0")
                    p8 = pmix.tile([128, NCHUNK], F32, tag="mix", name="po8")
                    nc.tensor.matmul(p0[:, :nw],
                                     lhsT=OT01[0:64, ts:ts + 128],
                                     rhs=wo01[0:64, n0:n0 + nw],
                                     start=True, stop=not even)
                    if even:
                        nc.tensor.matmul(p0[:, :nw],
                                         lhsT=OT2[0:64, ts:ts + 128],
                                         rhs=wo2[0:64, n0:n0 + nw],
                                         start=False, stop=True)
                    nc.tensor.matmul(p8[:, :nw],
                                     lhsT=OT01[64:128, ts:ts + 128],
                                     rhs=wo01[64:128, n0:n0 + nw],
                                     start=True, stop=even)
                    if not even:
                        nc.tensor.matmul(p8[:, :nw],
                                         lhsT=OT2b[64:128, ts:ts + 128],
                                         rhs=wo2b[64:128, n0:n0 + nw],
                                         start=False, stop=True)
                    nc.vector.tensor_copy(ob[:, n0:n0 + nw], p0[:, :nw])
                    nc.vector.tensor_add(ob[:, n0:n0 + nw],
                                         ob[:, n0:n0 + nw], p8[:, :nw])
                nc.sync.dma_start(out_d[ts:ts + 128, :], ob)

            def interleave(a, b):
                """Alternate thunks from two lists (a first), draining both."""
                ia = ib = 0
                while ia < len(a) or ib < len(b):
                    if ia < len(a):
                        a[ia](); ia += 1
                    if ib < len(b):
                        b[ib](); ib += 1

            # ================= emission =================
            # QK projection for heads 0/1 (m=0: [Qh0|Qh1], m=1: [Kh0|Kh1]),
            # n0-outer so the first four units only need the xT first-half
            # DMAs — the PE pipeline starts while the second halves stream
            for n0 in range(0, T, NCHUNK):
                for m in range(2):
                    proj_unit(m, n0)
            load_wv()

            # first score pairs (starts ACT exp pipeline early)
            sc01 = [[
                (lambda kj=kj, blk=blk: sc01_unit(kj, blk))
                for kj in range(4 * qw, 4 * qw + 4) for blk in _blocks(kj)]
                for qw in range(NQW)]
            for f in sc01[0][:2]:
                f()

            # m2 projection ([Qh2|Kh2]) then the head2 operand re-bases
            for n0 in range(0, T, NCHUNK):
                proj_unit(2, n0)
            nc.sync.dma_start(kt2, mt[2][64:128, :])
            nc.sync.dma_start(qk2b[64:128, :], mt[2][0:64, :])

            sc2 = sc2_units()
            vproj = [(lambda j=j: vproj_unit(j)) for j in range(NKJ)]

            # PE filler is budgeted against the per-window exp backlog
            # (ACT needs ~19/15/10/4.5us for windows 0-3): 8 vproj units in
            # phase 0, the rest + the PV/o_proj ladder cover phases 1-3,
            # with ops(qw) laced one window late and ops(2) pulled forward
            # so the tail is only pvns(3)+ops(3)
            interleave(sc01[0][2:] + sc2[0], vproj[:8])
            load_wo()  # first read at ops(0); keeps it off the rebase DMAs

            def pvns(qw):
                th = []
                for h in range(HPC):
                    def pvn(h=h, qw=qw):
                        acc = pv_chain(h, qw)
                        norm_unit(h, qw, acc)
                    th.append(pvn)
                return th

            def ops(qw):
                return [(lambda tt=tt: op_unit(tt))
                        for tt in range(4 * qw, 4 * qw + 4)]

            # op(qw) rides one ladder step late so the recip/broadcast/
            # normalize chain hides behind the next window's PV chains
            def lace(pv_th, op_th):
                out = []
                for i in range(len(pv_th)):
                    out.append(pv_th[i])
                    if i < len(op_th):
                        out.append(op_th[i])
                out += op_th[len(pv_th):]
                return out

            # Cascade: each window's PV rides in the SAME phase as its
            # scores (queued after them — pv chains before their score
            # units would deadlock the PE FIFO), ops one phase later.
            # Every phase's PE budget then covers its exp backlog, and the
            # tail is a short dense ops block.
            interleave(sc01[1] + sc2[1], vproj[8:] + pvns(0) + pvns(1))
            interleave(sc01[2] + sc2[2], ops(0) + pvns(2))
            interleave(sc01[3] + sc2[3], ops(1) + pvns(3))
            for f in ops(2) + ops(3):
                f()

    nc.compile()
    return nc


def _get_program():
    if "nc" not in _CACHE:
        _CACHE["nc"] = _build_program()
    return _CACHE["nc"]


def _shard_inputs(x, Wqkv, Wo):
    """Build the 8 per-core input maps."""
    in_maps = []
    for c in range(N_CORES):
        b, hg = divmod(c, GROUPS)
        h0 = HPC * hg
        def qcol(h):
            return Wqkv[:, (h0 + h) * HD:(h0 + h + 1) * HD]
        def kcol(h):
            return Wqkv[:, D + (h0 + h) * HD:D + (h0 + h + 1) * HD]
        def vcol(h):
            return Wqkv[:, 2 * D + (h0 + h) * HD:2 * D + (h0 + h + 1) * HD]
        # mt0=[Qh0|Qh1] mt1=[Kh0|Kh1] mt2=[Qh2|Kh2]
        wqk = np.concatenate([qcol(0), qcol(1), kcol(0), kcol(1),
                              qcol(2), kcol(2)], axis=1)
        wv = np.concatenate([vcol(0), vcol(1), vcol(2)], axis=1)
        in_maps.append({
            "xT": np.ascontiguousarray(x[b].T).astype(NP_IN),
            "wqk": np.ascontiguousarray(wqk).astype(NP_IN),
            "wv": np.ascontiguousarray(wv).astype(NP_IN),
            "wo": np.ascontiguousarray(
                Wo[h0 * HD:(h0 + HPC) * HD, :]).astype(NP_IN),
        })
    return in_maps


def kernel(x, attn_mask, Wqkv, Wo, bo):
    x = np.asarray(x, dtype=np.float32)
    Wqkv = np.asarray(Wqkv, dtype=np.float32)
    Wo = np.asarray(Wo, dtype=np.float32)
    bo = np.asarray(bo, dtype=np.float32)
    # attn_mask is causal by construction; causality is hardcoded on-device.

    nc = _get_program()
    in_maps = _shard_inputs(x, Wqkv, Wo)

    res = run_bass_kernel_spmd(nc, in_maps, core_ids=list(range(N_CORES)),
                               **_CACHE.get("run_kwargs", {}))
    _CACHE["last_results"] = res

    out = np.zeros((B, T, D), dtype=np.float32)
    for c in range(N_CORES):
        b = c // GROUPS
        out[b] += res.results[c]["out"]
    out += bo[None, None, :]
    return out


# revision 41
# speedup vs baseline: 1.0281x; 1.0281x over previous
"""Trainium2 Bass kernel for causal multi-head self-attention.

Problem: nn_MultiHeadSelfAttention (B=2, T=2048, D=768, H=12, HD=64).

    qkv = x @ Wqkv ; per-head causal softmax(q k^T / sqrt(hd)) @ v ; out @ Wo + bo

Sharding (8 cores): data-parallel over B (2) x tensor-parallel over heads
(4 groups of 3 heads).  Each core computes the QKV projection for its own
head slice, runs attention for its 3 heads, and produces a partial o_proj
output [T, D] (rows of Wo for its heads).  Host sums the 4 partials per
batch and adds the bias.

v2 design notes (HAM-warmth + PE row-tiling):
  - Scores (K=64 contraction) are emitted as row-tile PAIRS: head0 on PE
    rows 0:63 and head1 on rows 64:127 run concurrently (the bass
    auto-derived tile_position comes straight from the operand base
    partitions, which the [Qh0|Qh1]/[Kh0|Kh1] projection layout already
    provides).  Head2 pairs even-kj (operands re-based to partitions 0:63)
    with odd-kj (operands at partitions 64:127 via one SBUF->SBUF DMA).
  - P^T for all 3 heads persists in SBUF (bf16), so PV runs q-window-major:
    one PSUM bank per (head, window) accumulates over all its key tiles and
    is drained immediately.  o_proj for a window runs as soon as its 3 OT
    slices land, spreading the output DMA across the kernel.
  - The softmax division is applied to OT itself: the denominator row is
    inverted (DVE), partition-broadcast (GPSIMD), and multiplied into the
    PSUM->SBUF drain, so o_proj is a pure PSUM accumulation (head0+head2
    chained on PE rows 0:63 concurrent with head1(+head2 odd) on 64:127)
    and the per-head scalar_tensor_tensor epilogue disappears.
  - exp is the ACT-engine bottleneck (~52k columns); scores emission is
    interleaved with the m2/V projections so the PE always has independent
    work while ACT drains the score PSUM tiles (keeps the HAM clock gate
    at K=8/8 instead of the re-throttled 1.2 GHz the v1 kernel sat at).
"""

import os
import sys

for _p in ("/opt/trn_rl_repo",):
    if os.path.isdir(_p) and _p not in sys.path:
        sys.path.insert(0, _p)

import numpy as np
import ml_dtypes

import concourse.bass as bass
import concourse.mybir as mybir
import concourse.tile as tile
from concourse import bacc
from concourse.bass_utils import run_bass_kernel_spmd
from concourse.masks import make_lower_triangular

F32 = mybir.dt.float32

MM_MODE = os.environ.get("MM_DT", "bf16")
if MM_MODE == "f32r":
    MM_DT = mybir.dt.float32r
    NP_IN = np.float32
else:
    MM_DT = mybir.dt.bfloat16
    NP_IN = ml_dtypes.bfloat16

B, T, D, H = 2, 2048, 768, 12
HD = 64
HPC = 3            # heads per core
GROUPS = 4         # head groups (tensor-parallel)
N_CORES = 8
KT = D // 128      # 6 k-tiles over the feature dim
QKCOLS = HPC * 2 * HD  # 384 projected q/k columns
VC = HPC * HD          # 192 v columns
SCALE = 1.0 / np.sqrt(HD)
NCHUNK = 512
NQW = T // NCHUNK  # 4 query windows
NKJ = T // 128     # 16 key tiles
VBW = HD + 2       # v block width incl. ones columns
NEG = -1.0e30

# P^T column offset per key tile (ragged causal extents packed back to back)
EXT = [T - 128 * kj for kj in range(NKJ)]
OFF = [2048 * kj - 64 * kj * (kj - 1) for kj in range(NKJ)]
PTW = OFF[NKJ - 1] + EXT[NKJ - 1]  # 17408


def _chunks(kj):
    """(qstart, width) score chunks for key tile kj (causal: q >= 128*kj)."""
    out = []
    q = 128 * kj
    while q < T:
        w = min(NCHUNK, T - q)
        out.append((q, w))
        q += w
    return out


_CACHE = {}


def _one_act_table():
    """Make the act-table-load pass resolve both Exp and Ln to the single
    set that holds them both (natural_log_exp_and_others).  The pass picks
    a canonical set per function, so a kernel mixing exp and ln otherwise
    thrashes ACT_TABLE_LOADs (~1.3us each).  Only the python-side
    membership view is narrowed — set ids keep their act_info.json order,
    and the chosen set genuinely contains both functions."""
    import concourse.hw_specs as hw_specs
    if _CACHE.get("act_patched"):
        return
    orig = hw_specs.get_activation_tables

    def patched(arch):
        t = {name: set(fn) for name, fn in orig(arch).items()}
        both = {mybir.ActivationFunctionType.Exp,
                mybir.ActivationFunctionType.Ln}
        keep = "natural_log_exp_and_others"
        if keep in t and both <= t[keep]:
            for name, fns in t.items():
                if name != keep:
                    fns -= both
        return t

    hw_specs.get_activation_tables = patched
    bacc.get_activation_tables = patched
    _CACHE["act_patched"] = True


def _build_program():
    _one_act_table()
    nc = bacc.Bacc("TRN2", target_bir_lowering=False, debug=False,
                   num_devices=N_CORES, name="mhsa")

    xT_d = nc.dram_tensor("xT", [D, T], MM_DT, kind="ExternalInput").ap()
    wqk_d = nc.dram_tensor("wqk", [D, QKCOLS], MM_DT, kind="ExternalInput").ap()
    wv_d = nc.dram_tensor("wv", [D, VC], MM_DT, kind="ExternalInput").ap()
    wo_d = nc.dram_tensor("wo", [VC, D], MM_DT, kind="ExternalInput").ap()
    # output stored bf16 (halves the 6.3MB store stream; host sums in f32)
    out_d = nc.dram_tensor("out", [T, D], MM_DT, kind="ExternalOutput").ap()

    with tile.TileContext(nc) as tc:
        with (
            tc.tile_pool(name="const", bufs=1) as const,
            tc.tile_pool(name="persist", bufs=1) as persist,
            tc.tile_pool(name="obp", bufs=3) as obp,
            tc.tile_pool(name="nrm", bufs=2) as nrm,
            tc.tile_pool(name="psc", bufs=2, space="PSUM") as psc,
            tc.tile_pool(name="pmix", bufs=2, space="PSUM") as pmix,
            tc.tile_pool(name="pacc", bufs=2, space="PSUM") as pacc,
        ):
            # ---- constants ----
            maskneg = const.tile([128, 128], F32, tag="maskneg")
            make_lower_triangular(nc, maskneg, val=NEG, diag=False)
            ones_f = const.tile([128, 2], F32, tag="ones_f")
            nc.gpsimd.memset(ones_f, 1.0)
            ones_t = const.tile([128, 2], MM_DT, tag="ones_t")
            nc.vector.tensor_copy(ones_t, ones_f)

            # ---- input tiles (k-interleaved so matmul k=0 starts early) ----
            # xT loads are split in column halves: all first halves (plus
            # wqk) land in ~4.5us, so the n0-outer projection below starts
            # ~6us earlier than waiting for the full 3.5MB input stream
            xT_t, wqk_t = [], []
            for k in range(KT):
                xt = persist.tile([128, T], MM_DT, tag=f"xT{k}")
                nc.sync.dma_start(xt[:, 0:T // 2],
                                  xT_d[k * 128:(k + 1) * 128, 0:T // 2])
                xT_t.append(xt)
                wt = persist.tile([128, QKCOLS], MM_DT, tag=f"wqk{k}")
                nc.sync.dma_start(wt, wqk_d[k * 128:(k + 1) * 128, :])
                wqk_t.append(wt)
            for k in range(KT):
                nc.sync.dma_start(xT_t[k][:, T // 2:T],
                                  xT_d[k * 128:(k + 1) * 128, T // 2:T])
            # wv/wo tiles allocated here; their loads are emitted after the
            # m01 projection DMAs so xT/wqk win the queue and matmul 0
            # starts sooner (wv is first read mid-phase-0, wo at ops(0))
            wv_t = []
            for k in range(KT):
                wt = persist.tile([128, VC], MM_DT, tag=f"wv{k}", name=f"wv{k}")
                wv_t.append(wt)
            # o_proj weights: h0 rows on partitions 0:64, h1 on 64:128,
            # h2 on both 0:64 (wo2, even token tiles) and 64:128 (wo2b, odd)
            wo01 = persist.tile([128, D], MM_DT, tag="wo01")
            wo2 = persist.tile([64, D], MM_DT, tag="wo2")
            wo2b = persist.tile([128, D], MM_DT, tag="wo2b")

            def load_wv():
                for k in range(KT):
                    nc.sync.dma_start(wv_t[k], wv_d[k * 128:(k + 1) * 128, :])

            def load_wo():
                nc.sync.dma_start(wo01[0:64, :], wo_d[0:HD, :])
                nc.sync.dma_start(wo01[64:128, :], wo_d[HD:2 * HD, :])
                nc.sync.dma_start(wo2, wo_d[2 * HD:3 * HD, :])
                nc.sync.dma_start(wo2b[64:128, :], wo_d[2 * HD:3 * HD, :])

            # ---- persistent intermediates ----
            mt = [persist.tile([128, T], MM_DT, tag=f"mt{m}", name=f"mt{m}")
                  for m in range(3)]
            kt2 = persist.tile([64, T], MM_DT, tag="kt2")     # Kh2 -> base 0
            qk2b = persist.tile([128, T], MM_DT, tag="qk2b")  # Qh2 -> base 64
            # V blocks: [128, (j, h, VBW)] — one strided copy lands all 3
            # heads of a token tile; ones columns at 64:66 of each block
            V_all = persist.tile([128, NKJ * HPC * VBW], MM_DT, tag="V")
            va3 = V_all.rearrange("p (b c) -> p b c", c=VBW)
            nc.vector.tensor_copy(
                va3[:, :, HD:HD + 2],
                ones_t.unsqueeze(1).to_broadcast((128, NKJ * HPC, 2)))
            PT = [persist.tile([128, PTW], MM_DT, tag=f"PT{h}", name=f"PT{h}")
                  for h in range(HPC)]
            OT01 = persist.tile([128, T], MM_DT, tag="OT01", name="OT01")
            OT2 = persist.tile([64, T], MM_DT, tag="OT2", name="OT2")
            OT2b = persist.tile([128, T], MM_DT, tag="OT2b", name="OT2b")

            # ---- unit emitters ----
            def proj_unit(m, n0):
                ps = pmix.tile([128, NCHUNK], F32, tag="mix", name="proj")
                for k in range(KT):
                    nc.tensor.matmul(
                        ps, lhsT=wqk_t[k][:, m * 128:(m + 1) * 128],
                        rhs=xT_t[k][:, n0:n0 + NCHUNK],
                        start=(k == 0), stop=(k == KT - 1))
                nc.vector.tensor_copy(mt[m][:, n0:n0 + NCHUNK], ps)

            def vproj_unit(j):
                ps = pmix.tile([128, NCHUNK], F32, tag="mix", name="vproj")
                for k in range(KT):
                    nc.tensor.matmul(
                        ps[:, :VC], lhsT=xT_t[k][:, j * 128:(j + 1) * 128],
                        rhs=wv_t[k], start=(k == 0), stop=(k == KT - 1))
                v3 = V_all[:, j * HPC * VBW:(j + 1) * HPC * VBW].rearrange(
                    "p (h c) -> p h c", c=VBW)
                p3 = ps[:, :VC].rearrange("p (h c) -> p h c", c=HD)
                nc.vector.tensor_copy(v3[:, :, 0:HD], p3)

            def sc_block(h, lhsT, rhs_src, lo, kj, blk):
                """One exp block for head h / key tile kj: up to 2 score
                matmuls (bank-aligned halves of a 2-bank PSUM tile) drained
                by a single ACTIVATE — halves the ACT per-instr overhead."""
                wtot = sum(w for _, w in blk)
                ps = psc.tile([128, 2 * NCHUNK], F32, tag="sc", name="sc")
                off = 0
                for q, w in blk:
                    nc.tensor.matmul(ps[:, off:off + w],
                                     lhsT=lhsT[lo:lo + 64,
                                               kj * 128:kj * 128 + 128],
                                     rhs=rhs_src[lo:lo + 64, q:q + w],
                                     start=True, stop=True)
                    if q == 128 * kj:
                        nc.vector.tensor_add(ps[:, off:off + 128],
                                             ps[:, off:off + 128], maskneg)
                    off += w
                c = OFF[kj] + (blk[0][0] - 128 * kj)
                nc.scalar.activation(PT[h][:, c:c + wtot], ps[:, :wtot],
                                     mybir.ActivationFunctionType.Exp,
                                     scale=float(SCALE))

            def _blocks(kj):
                ch = _chunks(kj)
                return [ch[i:i + 2] for i in range(0, len(ch), 2)]

            def sc01_unit(kj, blk):
                # head0 on PE rows 0:63, head1 on rows 64:127 (concurrent)
                sc_block(0, mt[1], mt[0], 0, kj, blk)
                sc_block(1, mt[1], mt[0], 64, kj, blk)

            def sc2_units():
                """Head2 score pair units: even kj at rows 0:63 (kt2/mt2),
                odd kj at rows 64:127 (mt2/qk2b), block i with block i."""
                wins = [[] for _ in range(NQW)]
                for kp in range(0, NKJ, 2):
                    ba, bb = _blocks(kp), _blocks(kp + 1)
                    for i in range(max(len(ba), len(bb))):
                        def emit(kp=kp, i=i, ba=ba, bb=bb):
                            if i < len(ba):
                                sc_block(2, kt2, mt[2], 0, kp, ba[i])
                            if i < len(bb):
                                sc_block(2, mt[2], qk2b, 64, kp + 1, bb[i])
                        wins[kp // 4].append(emit)
                return wins

            def pv_chain(h, qw):
                acc = pacc.tile([128, NCHUNK], F32, tag="acc", name="acc")
                q0 = NCHUNK * qw
                last = 4 * qw + 3
                for kj in range(last + 1):
                    lo = max(0, 128 * kj - q0)
                    c = OFF[kj] + (q0 + lo - 128 * kj)
                    vb = (kj * HPC + h) * VBW
                    nc.tensor.matmul(
                        acc[:VBW, lo:NCHUNK],
                        lhsT=V_all[:, vb:vb + VBW],
                        rhs=PT[h][:, c:c + (NCHUNK - lo)],
                        start=(kj == 0), stop=(kj == last))
                return acc

            def norm_unit(h, qw, acc):
                """OT[:, window] = acc[0:64] / denominator-row (pre-divided
                so o_proj can accumulate heads directly in PSUM)."""
                # Stage acc (attention rows + denominator row) out of PSUM
                # right away so the bank frees without waiting on the recip
                # chain.  1/d runs on ACT as exp(-ln d) — the DVE reciprocal
                # is ~6.5ns/element (3.3us per row) and the custom-DVE
                # reciprocal_approx_fast produces garbage on this toolchain.
                # _one_act_table() keeps ln+exp in a single table set.
                stage = nrm.tile([65, NCHUNK], F32, tag="stage")
                nc.vector.tensor_copy(stage, acc[0:65, :])
                rl = nrm.tile([1, NCHUNK], F32, tag="rl")
                nc.scalar.activation(rl, stage[64:65, :],
                                     mybir.ActivationFunctionType.Ln)
                rr = nrm.tile([1, NCHUNK], F32, tag="rr")
                nc.scalar.activation(rr, rl,
                                     mybir.ActivationFunctionType.Exp,
                                     scale=-1.0)
                rb = nrm.tile([64, NCHUNK], F32, tag="rb")
                nc.gpsimd.partition_broadcast(rb, rr, channels=64)
                qs = qw * NCHUNK
                if h == 0:
                    nc.vector.tensor_mul(OT01[0:64, qs:qs + NCHUNK],
                                         stage[0:64, :], rb)
                elif h == 1:
                    st = nrm.tile([64, NCHUNK], MM_DT, tag="st")
                    nc.vector.tensor_mul(st, stage[0:64, :], rb)
                    nc.sync.dma_start(OT01[64:128, qs:qs + NCHUNK], st)
                else:
                    nc.vector.tensor_mul(OT2[:, qs:qs + NCHUNK],
                                         stage[0:64, :], rb)
                    nc.sync.dma_start(OT2b[64:128, qs:qs + NCHUNK],
                                      OT2[:, qs:qs + NCHUNK])

            def op_unit(tt):
                ob = obp.tile([128, D], MM_DT, tag="ob")
                ts = tt * 128
                even = (tt % 2 == 0)
                for n0, nw in ((0, 512), (512, 256)):
                    p0 = pmix.tile([128, NCHUNK], F32, tag="mix", name="po0")
                    p8 = pmix.tile([128, NCHUNK], F32, tag="mix", name="po8")
                    nc.tensor.matmul(p0[:, :nw],
                                     lhsT=OT01[0:64, ts:ts + 128],
                                     rhs=wo01[0:64, n0:n0 + nw],
                                     start=True, stop=not even)
                    if even:
                        nc.tensor.matmul(p0[:, :nw],
                                         lhsT=OT2[0:64, ts:ts + 128],
                                         rhs=wo2[0:64, n0:n0 + nw],
                                         start=False, stop=True)
                    nc.tensor.matmul(p8[:, :nw],
                                     lhsT=OT01[64:128, ts:ts + 128],
                                     rhs=wo01[64:128, n0:n0 + nw],
                                     start=True, stop=even)
                    if not even:
                        nc.tensor.matmul(p8[:, :nw],
                                         lhsT=OT2b[64:128, ts:ts + 128],
                                         rhs=wo2b[64:128, n0:n0 + nw],
                                         start=False, stop=True)
                    nc.vector.tensor_copy(ob[:, n0:n0 + nw], p0[:, :nw])
                    nc.vector.tensor_add(ob[:, n0:n0 + nw],
                                         ob[:, n0:n0 + nw], p8[:, :nw])
                nc.sync.dma_start(out_d[ts:ts + 128, :], ob)

            def interleave(a, b):
                """Alternate thunks from two lists (a first), draining both."""
                ia = ib = 0
                while ia < len(a) or ib < len(b):
                    if ia < len(a):
                        a[ia](); ia += 1
                    if ib < len(b):
                        b[ib](); ib += 1

            # ================= emission =================
            # QK projection for heads 0/1 (m=0: [Qh0|Qh1], m=1: [Kh0|Kh1]),
            # n0-outer so the first four units only need the xT first-half
            # DMAs — the PE pipeline starts while the second halves stream
            for n0 in range(0, T, NCHUNK):
                for m in range(2):
                    proj_unit(m, n0)
            load_wv()

            # first score pairs (starts ACT exp pipeline early)
            sc01 = [[
                (lambda kj=kj, blk=blk: sc01_unit(kj, blk))
                for kj in range(4 * qw, 4 * qw + 4) for blk in _blocks(kj)]
                for qw in range(NQW)]
            for f in sc01[0][:2]:
                f()

            # m2 projection ([Qh2|Kh2]) then the head2 operand re-bases
            for n0 in range(0, T, NCHUNK):
                proj_unit(2, n0)
            nc.sync.dma_start(kt2, mt[2][64:128, :])
            nc.sync.dma_start(qk2b[64:128, :], mt[2][0:64, :])

            sc2 = sc2_units()
            vproj = [(lambda j=j: vproj_unit(j)) for j in range(NKJ)]

            # PE filler is budgeted against the per-window exp backlog
            # (ACT needs ~19/15/10/4.5us for windows 0-3): 8 vproj units in
            # phase 0, the rest + the PV/o_proj ladder cover phases 1-3,
            # with ops(qw) laced one window late and ops(2) pulled forward
            # so the tail is only pvns(3)+ops(3)
            interleave(sc01[0][2:] + sc2[0], vproj[:8])
            load_wo()  # first read at ops(0); keeps it off the rebase DMAs

            def pvns(qw):
                th = []
                for h in range(HPC):
                    def pvn(h=h, qw=qw):
                        acc = pv_chain(h, qw)
                        norm_unit(h, qw, acc)
                    th.append(pvn)
                return th

            def ops(qw):
                return [(lambda tt=tt: op_unit(tt))
                        for tt in range(4 * qw, 4 * qw + 4)]

            # op(qw) rides one ladder step late so the recip/broadcast/
            # normalize chain hides behind the next window's PV chains
            def lace(pv_th, op_th):
                out = []
                for i in range(len(pv_th)):
                    out.append(pv_th[i])
                    if i < len(op_th):
                        out.append(op_th[i])
                out += op_th[len(pv_th):]
                return out

            interleave(sc01[1] + sc2[1], vproj[8:] + pvns(0))
            interleave(sc01[2] + sc2[2], lace(pvns(1), ops(0)))
            interleave(sc01[3] + sc2[3], lace(pvns(2), ops(1)) + ops(2))
            for f in pvns(3) + ops(3):
                f()

    nc.compile()
    return nc


def _get_program():
    if "nc" not in _CACHE:
        _CACHE["nc"] = _build_program()
    return _CACHE["nc"]


def _shard_inputs(x, Wqkv, Wo):
    """Build the 8 per-core input maps."""
    in_maps = []
    for c in range(N_CORES):
        b, hg = divmod(c, GROUPS)
        h0 = HPC * hg
        def qcol(h):
            return Wqkv[:, (h0 + h) * HD:(h0 + h + 1) * HD]
        def kcol(h):
            return Wqkv[:, D + (h0 + h) * HD:D + (h0 + h + 1) * HD]
        def vcol(h):
            return Wqkv[:, 2 * D + (h0 + h) * HD:2 * D + (h0 + h + 1) * HD]
        # mt0=[Qh0|Qh1] mt1=[Kh0|Kh1] mt2=[Qh2|Kh2]
        wqk = np.concatenate([qcol(0), qcol(1), kcol(0), kcol(1),
                              qcol(2), kcol(2)], axis=1)
        wv = np.concatenate([vcol(0), vcol(1), vcol(2)], axis=1)
        in_maps.append({
            "xT": np.ascontiguousarray(x[b].T).astype(NP_IN),
            "wqk": np.ascontiguousarray(wqk).astype(NP_IN),
            "wv": np.ascontiguousarray(wv).astype(NP_IN),
            "wo": np.ascontiguousarray(
                Wo[h0 * HD:(h0 + HPC) * HD, :]).astype(NP_IN),
        })
    return in_maps


def kernel(x, attn_mask, Wqkv, Wo, bo):
    x = np.asarray(x, dtype=np.float32)
    Wqkv = np.asarray(Wqkv, dtype=np.float32)
    Wo = np.asarray(Wo, dtype=np.float32)
    bo = np.asarray(bo, dtype=np.float32)
    # attn_mask is causal by construction; causality is hardcoded on-device.

    nc = _get_program()
    in_maps = _shard_inputs(x, Wqkv, Wo)

    res = run_bass_kernel_spmd(nc, in_maps, core_ids=list(range(N_CORES)),
                               **_CACHE.get("run_kwargs", {}))
    _CACHE["last_results"] = res

    out = np.zeros((B, T, D), dtype=np.float32)
    for c in range(N_CORES):
        b = c // GROUPS
        out[b] += np.asarray(res.results[c]["out"], dtype=np.float32)
    out += bo[None, None, :]
    return out
